# revision 5
# baseline (speedup 1.0000x reference)
"""Trainium2 Bass kernel for nn_MEGNet_State_876173328941.

MEGNet state update: u_e = scatter_mean(edge_attr, batch[edge_index[0]], B),
u_v = scatter_mean(x, batch, B), comb = [u_e, u_v, state], 3-layer MLP
(96->32->32->32) with training-mode BatchNorm over the 1024 graphs.

Strategy (868us baseline -> ~143us, DMA-roofline bound):
  - Rows are quantized to fp8 e4m3 on the host (DMA traffic /4 vs f32).
    Each row is pre-scaled by r_g = 1/(max(cnt_g,1) * s_type) so the device
    computes plain sums; the power-of-two s_type is applied in the psum
    drains. Measured final rel err ~3.3e-3 vs the 2e-2 gate.
  - Per core, its 128 graphs are ranked by edge-tile count and grouped into
    8 groups of 16 lanes. A slab = [128 rows, 512 cols] where col = f*16+lane
    (feat-major). One matmul with stationary ones[128,1] reduces a slab into
    psum bank g's row [1, 512] (all 16 graphs x 32 feats at once); slabs of a
    group accumulate via start/stop. fp8 DoubleRow processes 2 slabs (256
    rows) per matmul. No per-tile weight reloads, N=512 per matmul.
  - Segment sums drain psum->SBUF fp16, 2 small DMAs -> AllGather
    [2,4096] -> [16,4096] (warmed up by an early dummy collective), then
    rearranged gather DMAs assemble comb [96, 1024] fp16.
  - fp16 MLP in [feat, graph] layout; BatchNorm via bn_stats/bn_aggr with
    single-engine dependency chains; dummy matmuls keep the PE HAM-warm
    across the collective gap.
"""

import sys

sys.path.insert(0, "/opt/trn_rl_repo")

import ml_dtypes
import numpy as np

import concourse.bacc as bacc
import concourse.tile as tile
from concourse import mybir
from concourse.bass_utils import run_bass_kernel_spmd

F8 = ml_dtypes.float8_e4m3   # TRN fp8e4 (max normal 240)
DIM = 32
B = 1024
N_CORES = 8
SEGS = 128          # graphs per core
GROUPS = 8          # psum banks / lane-groups per core
LANES = 16          # graphs per group (16*32 feats = 512 = one psum bank)
SLAB = 512          # columns per slab
EPS = 1e-5
USE_DR = True       # fp8 DoubleRow (2 slabs / matmul)
# ramped chunk sizes (slabs): small first so matmuls start early, then 2MB
CHUNK_RAMP = [8, 16]
CHUNK_MAX = 24
WARM_MMS = 90       # fp16 N=512 dummy matmuls to keep HAM warm across the AG

_CACHE = {}


# ---------------------------------------------------------------- host planning

def _plan(ecnt, ncnt):
    """LPT graph->core assignment + shared (max over cores) group schedules."""
    e_t = np.maximum((ecnt + 127) // 128, 1).astype(np.int64)
    n_t = np.maximum((ncnt + 127) // 128, 1).astype(np.int64)

    order_desc = np.argsort(-e_t, kind="stable")
    load = np.zeros(N_CORES, dtype=np.int64)
    nseg = np.zeros(N_CORES, dtype=np.int64)
    assign = np.zeros(B, dtype=np.int64)
    for s in order_desc:
        open_cores = np.where(nseg < SEGS)[0]
        k = open_cores[np.argmin(load[open_cores])]
        assign[s] = k
        load[k] += e_t[s]
        nseg[k] += 1

    order = np.zeros((N_CORES, SEGS), dtype=np.int64)   # rank -> global seg
    rank_of = np.zeros(B, dtype=np.int64)
    for k in range(N_CORES):
        segs_k = np.where(assign == k)[0]
        segs_k = segs_k[np.argsort(-e_t[segs_k], kind="stable")]
        order[k] = segs_k
        rank_of[segs_k] = np.arange(SEGS)

    def sched(tiles):
        per_rank = tiles[order].max(axis=0)                 # [SEGS]
        return per_rank.reshape(GROUPS, LANES).max(axis=1)  # [GROUPS]

    sched_e = sched(e_t)
    sched_n = sched(n_t)
    p_global = order.reshape(-1)        # gathered col j -> original seg id
    return assign, rank_of, sched_e, sched_n, p_global


def _pack(rows, seg, cnt, assign, rank_of, sched, base_slab, data):
    """Scatter fp8 rows into the per-core [128, W] streams.

    data: [N_CORES, 128, W] fp8 array (written in place).
    Slab s of a group g sits at columns [ (base_slab+off_g+s)*512 , +512 ),
    col = f*16 + lane. Row w of its graph -> slab w//128, partition w%128.
    """
    off = np.zeros(GROUPS + 1, dtype=np.int64)
    np.cumsum(sched, out=off[1:])

    M = rows.shape[0]
    order = np.argsort(seg, kind="stable")
    srows = rows[order]
    sseg = seg[order]
    offs = np.zeros(B, dtype=np.int64)
    np.cumsum(cnt[:-1], out=offs[1:])
    within = np.arange(M, dtype=np.int64) - offs[sseg]

    core = assign[sseg]
    rank = rank_of[sseg]
    grp, lane = rank >> 4, rank & 15
    slab = base_slab + off[grp] + (within >> 7)
    part = within & 127
    # view: [core, partition, slab, f, lane]
    W = data.shape[2]
    v = data.reshape(N_CORES, 128, W // SLAB, DIM, LANES)
    v[core, part, slab, :, lane] = srows


# ---------------------------------------------------------------- device program

def _chunk_sizes(n_slabs):
    sizes = []
    left = n_slabs
    for c in CHUNK_RAMP:
        if left <= 0:
            break
        c = min(c, left)
        c -= c % 2
        if c:
            sizes.append(c)
            left -= c
    while left > 0:
        c = min(CHUNK_MAX, left)
        c -= c % 2
        if c == 0:
            c = left
        sizes.append(c)
        left -= c
    return sizes


def _build_nc(sched_n, sched_e, n_slabs, s_e, s_n):
    nc = bacc.Bacc("TRN2", target_bir_lowering=False, debug=False,
                   enable_asserts=False, num_devices=N_CORES)
    f32 = mybir.dt.float32
    f16 = mybir.dt.float16
    f8 = mybir.dt.float8e4
    AF = mybir.ActivationFunctionType

    W = n_slabs * SLAB
    data = nc.declare_dram_parameter("data", [128, W], f8, isOutput=False)
    ones8 = nc.declare_dram_parameter("ones8", [128, DIM], f8, isOutput=False)
    stateT = nc.declare_dram_parameter("stateT", [DIM, B], f16, isOutput=False)
    W1 = nc.declare_dram_parameter("W1", [3 * DIM, DIM], f16, isOutput=False)
    W2 = nc.declare_dram_parameter("W2", [DIM, DIM], f16, isOutput=False)
    W3 = nc.declare_dram_parameter("W3", [DIM, DIM], f16, isOutput=False)
    # vecs columns: b1,g1,be1,b2,g2,be2,b3,g3,be3
    vecs = nc.declare_dram_parameter("vecs", [DIM, 9], f32, isOutput=False)
    out = nc.declare_dram_parameter("out", [DIM, B], f32, isOutput=True)

    ag_in = nc.dram_tensor("ag_in", [2, GROUPS * SLAB], f16)
    ag_out = nc.dram_tensor("ag_out", [2 * N_CORES, GROUPS * SLAB], f16,
                            addr_space="Shared")
    agw_in = nc.dram_tensor("agw_in", [1, 16], f32)
    agw_out = nc.dram_tensor("agw_out", [N_CORES, 16], f32, addr_space="Shared")

    # slab schedule: nodes first, then edges; per type group-major
    phases = [("n", sched_n), ("e", sched_e)]
    scales = {"n": s_n, "e": s_e}

    with tile.TileContext(nc) as tc:
        with tc.tile_pool(name="const", bufs=1) as const, \
             tc.tile_pool(name="chunks", bufs=4) as chunks, \
             tc.tile_pool(name="work", bufs=1) as work:

            # ones8 first on the sync queue (needed by the first matmul)
            ones = const.tile([128, DIM], f8)
            nc.sync.dma_start(out=ones, in_=ones8[:, :])
            # DoubleRow stationary: [128, 2, 1] with 16B step between the
            # two k-subtiles (walrus perf-mode AP constraint)
            ones_dr = ones.rearrange("p (k m) -> p k m", k=2)[:, :, 0:1]

            # warmup AllGather: boots ncfw + absorbs cross-core start skew
            # while stage 1 streams; keep it entirely on the GpSimd queue so
            # it cannot block the chunk-DMA queues (result unused)
            warm0 = const.tile([1, 16], f32)
            nc.vector.memset(warm0, 0.0)
            nc.gpsimd.dma_start(out=agw_in[:, :], in_=warm0)
            nc.gpsimd.collective_compute(
                "AllGather",
                mybir.AluOpType.bypass,
                replica_groups=[list(range(N_CORES))],
                ins=[agw_in[:, :]],
                outs=[agw_out[:, :]],
            )

            # MLP constants ride the scalar queue (not needed until late)
            w1s = const.tile([3 * DIM, DIM], f16)
            nc.scalar.dma_start(out=w1s, in_=W1[:, :])
            w2s = const.tile([DIM, DIM], f16)
            nc.scalar.dma_start(out=w2s, in_=W2[:, :])
            w3s = const.tile([DIM, DIM], f16)
            nc.scalar.dma_start(out=w3s, in_=W3[:, :])
            vs = const.tile([DIM, 9], f32)
            nc.scalar.dma_start(out=vs, in_=vecs[:, :])

            # zeros for HAM-warming dummy matmuls
            wzero = const.tile([128, 512], f16)
            nc.vector.memset(wzero, 0.0)

            epsc = const.tile([DIM, 1], f32)
            nc.vector.memset(epsc, EPS)

            # preload ACT function tables so no ACT_TABLE_LOAD lands mid-MLP
            warm = const.tile([1, 4], f32)
            nc.vector.memset(warm, 0.0)
            for fn in (AF.Relu, AF.Square, AF.Sqrt, AF.Identity):
                nc.scalar.activation(out=warm[0:1, 1:2], in_=warm[0:1, 0:1],
                                     func=fn)

            su = {"e": work.tile([1, GROUPS * SLAB], f16, tag="su_e", name="su_e"),
                  "n": work.tile([1, GROUPS * SLAB], f16, tag="su_n", name="su_n")}

            # ---- stage 1: streamed segment sums ----
            with tc.tile_pool(name="spsum", bufs=1, space="PSUM") as spsum:
                # items: (type, group, width 1|2, start, stop); DoubleRow
                # pairs never straddle a chunk boundary
                items = []
                for ph, sched in phases:
                    for g in range(GROUPS):
                        T = int(sched[g])
                        t = 0
                        while t < T:
                            wdt = 2 if (USE_DR and t + 1 < T) else 1
                            items.append((ph, g, wdt, t == 0, t + wdt == T))
                            t += wdt
                assert sum(it[2] for it in items) == n_slabs

                # pack items into chunks of ~CHUNK_MAX slabs (ramped start)
                targets = list(CHUNK_RAMP) + [CHUNK_MAX] * len(items)
                chunks_items = []
                cur, cur_slabs, ti = [], 0, 0
                for it in items:
                    cur.append(it)
                    cur_slabs += it[2]
                    if cur_slabs >= targets[ti]:
                        chunks_items.append((cur, cur_slabs))
                        cur, cur_slabs, ti = [], 0, ti + 1
                if cur:
                    chunks_items.append((cur, cur_slabs))

                ps = {}
                s = 0
                for cit, csz in chunks_items:
                    ct = chunks.tile([128, csz * SLAB], f8, tag="chunk",
                                     name="ct")
                    nc.sync.dma_start(out=ct, in_=data[:, s * SLAB:(s + csz) * SLAB])
                    j = 0
                    for ph, g, wdt, start, stop in cit:
                        key = (ph, g)
                        if key not in ps:
                            ps[key] = spsum.tile([1, SLAB], f32, tag=f"b{g}",
                                                 name=f"ps_{ph}{g}")
                        if wdt == 2:
                            rhs = ct[:, j * SLAB:(j + 2) * SLAB].rearrange(
                                "p (k n) -> p k n", k=2)
                            nc.tensor.matmul(
                                out=ps[key][:, :], lhsT=ones_dr, rhs=rhs,
                                start=start, stop=stop,
                                perf_mode=mybir.MatmulPerfMode.DoubleRow)
                        else:
                            nc.tensor.matmul(
                                out=ps[key][:, :], lhsT=ones[:, 0:1],
                                rhs=ct[:, j * SLAB:(j + 1) * SLAB],
                                start=start, stop=stop)
                        j += wdt
                        if stop:
                            # drain to SBUF (fp16) with the type scale folded
                            # in, rescattering bank cols f*16+l to core-wide
                            # feat-major cols f*128 + (g*16+l); the last few
                            # groups drain on DVE so ACT/DVE split the tail
                            dst = su[ph].rearrange("p (f j) -> p f j", f=DIM)[
                                :, :, g * LANES:(g + 1) * LANES]
                            src = ps.pop((ph, g)).rearrange(
                                "p (f l) -> p f l", f=DIM)
                            if ph == "e" and g >= GROUPS - 4:
                                nc.vector.tensor_scalar_mul(
                                    dst, src, float(scales[ph]))
                            else:
                                nc.scalar.activation(out=dst, in_=src,
                                                     func=AF.Copy,
                                                     scale=float(scales[ph]))
                            if ph == "n" and g == GROUPS - 1:
                                # node sums complete early; ship them now
                                nc.scalar.dma_start(out=ag_in[1:2, :],
                                                    in_=su["n"][0:1, :])
                    s += csz

            # ---- collective: share per-core sums ----
            nc.sync.dma_start(out=ag_in[0:1, :], in_=su["e"][0:1, :])
            nc.gpsimd.collective_compute(
                "AllGather",
                mybir.AluOpType.bypass,
                replica_groups=[list(range(N_CORES))],
                ins=[ag_in[:, :]],
                outs=[ag_out[:, :]],
            )

            # ---- assemble comb [96, 1024] fp16 directly ----
            comb16 = work.tile([3 * DIM, B], f16, tag="comb16")
            # state slice has no AG dependency; it lands early via scalar
            nc.scalar.dma_start(out=comb16[2 * DIM:3 * DIM, :], in_=stateT[:, :])
            agv = ag_out.rearrange("(c r) (f j) -> r f c j", r=2, f=DIM)
            nc.sync.dma_start(out=comb16[0:DIM, :], in_=agv[0])
            nc.scalar.dma_start(out=comb16[DIM:2 * DIM, :], in_=agv[1])

            # ---- MLP + BatchNorm, layout [feat, graph] ----
            # relu/stats/scale-shift all on DVE (bn_stats gives mean+var in
            # one pass); single ACT hop per layer for the sqrt
            with tc.tile_pool(name="mpsum", bufs=2, space="PSUM") as mpsum:
                # dummy matmuls keep the PE HAM-warm across the collective
                # gap (each ~213ns; they sit between stage-1 and MLP in the
                # PE FIFO and retire long before comb16 is ready)
                wps = mpsum.tile([1, 512], f32, tag="warmps")
                for _ in range(WARM_MMS):
                    nc.tensor.matmul(out=wps[:, :], lhsT=wzero[:, 0:1],
                                     rhs=wzero[:, :], start=True, stop=True)

                h = comb16
                for layer in range(3):
                    w = (w1s, w2s, w3s)[layer]
                    bcol = vs[:, 3 * layer:3 * layer + 1]
                    gcol = vs[:, 3 * layer + 1:3 * layer + 2]
                    becol = vs[:, 3 * layer + 2:3 * layer + 3]
                    last = layer == 2

                    ps_h = mpsum.tile([DIM, B], f32, tag="ps_h")
                    for half in range(2):
                        sl = slice(half * 512, (half + 1) * 512)
                        nc.tensor.matmul(out=ps_h[:, sl], lhsT=w[:, :],
                                         rhs=h[:, sl], start=True, stop=True)
                    # hl = relu(ps_h + b) (last layer: no relu)
                    hl = work.tile([DIM, B], f16, tag=f"h{layer}",
                                   name=f"h{layer}")
                    if last:
                        nc.vector.tensor_scalar(hl, ps_h, bcol, None,
                                                mybir.AluOpType.add)
                    else:
                        nc.vector.tensor_scalar(hl, ps_h, bcol, 0.0,
                                                mybir.AluOpType.add,
                                                mybir.AluOpType.max)
                    # mean/var via bn_stats (512-wide HW limit -> two calls)
                    st6 = work.tile([DIM, 2 * 6], f32, tag="st6")
                    nc.vector.bn_stats(st6[:, 0:6], hl[:, 0:512])
                    nc.vector.bn_stats(st6[:, 6:12], hl[:, 512:1024])
                    mv = work.tile([DIM, 2], f32, tag="mv")
                    nc.vector.bn_aggr(mv, st6)
                    # sd = sqrt(var + eps)  (single ACT hop)
                    sd = work.tile([DIM, 1], f32, tag="sd")
                    nc.scalar.activation(out=sd, in_=mv[:, 1:2], func=AF.Sqrt,
                                         bias=epsc)
                    # DVE: rstd, rg, mrg, be2, hb
                    rstd = work.tile([DIM, 1], f32, tag="rstd")
                    nc.vector.reciprocal(rstd, sd)
                    rg = work.tile([DIM, 1], f32, tag="rg")
                    nc.vector.tensor_tensor(rg, rstd, gcol, mybir.AluOpType.mult)
                    mrg = work.tile([DIM, 1], f32, tag="mrg")
                    nc.vector.tensor_tensor(mrg, mv[:, 0:1], rg,
                                            mybir.AluOpType.mult)
                    be2 = work.tile([DIM, 1], f32, tag="be2")
                    nc.vector.tensor_tensor(be2, becol, mrg,
                                            mybir.AluOpType.subtract)
                    hb = work.tile([DIM, B], f32 if last else f16,
                                   tag=f"hb{layer}", name=f"hb{layer}")
                    nc.vector.tensor_scalar(hb, hl, rg, be2,
                                            mybir.AluOpType.mult,
                                            mybir.AluOpType.add)
                    h = hb

                nc.sync.dma_start(out=out[:, :], in_=h)

    nc.compile()
    return nc


# ---------------------------------------------------------------- entry points

def run(inputs, trace=False, sim=False):
    x = np.asarray(inputs["x"], dtype=np.float32)
    edge_index = np.asarray(inputs["edge_index"]).astype(np.int64)
    edge_attr = np.asarray(inputs["edge_attr"], dtype=np.float32)
    state = np.asarray(inputs["state"], dtype=np.float32)
    batch = np.asarray(inputs["batch"]).astype(np.int64)

    eseg = batch[edge_index[0]]
    ecnt = np.bincount(eseg, minlength=B)
    ncnt = np.bincount(batch, minlength=B)

    assign, rank_of, sched_e, sched_n, p_global = _plan(ecnt, ncnt)
    n_slabs = int(sched_n.sum() + sched_e.sum())

    # power-of-two type scales keep fp8 rows O(1); folded into W1 rows
    s_e = 2.0 ** -round(float(np.log2(max(ecnt.mean(), 1.0))))
    s_n = 2.0 ** -round(float(np.log2(max(ncnt.mean(), 1.0))))
    r_e = 1.0 / (np.maximum(ecnt, 1.0) * s_e)
    r_n = 1.0 / (np.maximum(ncnt, 1.0) * s_n)

    erows = np.clip(edge_attr * r_e[eseg][:, None], -239.0, 239.0).astype(F8)
    nrows = np.clip(x * r_n[batch][:, None], -239.0, 239.0).astype(F8)

    data = np.zeros((N_CORES, 128, n_slabs * SLAB), dtype=F8)
    _pack(nrows, batch, ncnt, assign, rank_of, sched_n, 0, data)
    _pack(erows, eseg, ecnt, assign, rank_of, sched_e, int(sched_n.sum()), data)

    vecs = np.stack([np.asarray(inputs[k], np.float32) for k in
                     ("b1", "g1", "be1", "b2", "g2", "be2", "b3", "g3", "be3")],
                    axis=1)

    shared = {
        "ones8": np.ones((128, DIM), dtype=F8),
        "stateT": np.ascontiguousarray(state.T[:, p_global]).astype(np.float16),
        "W1": np.asarray(inputs["W1"], np.float16),
        "W2": np.asarray(inputs["W2"], np.float16),
        "W3": np.asarray(inputs["W3"], np.float16),
        "vecs": np.ascontiguousarray(vecs),
    }
    in_maps = []
    for k in range(N_CORES):
        m = dict(shared)
        m["data"] = np.ascontiguousarray(data[k])
        in_maps.append(m)

    key = (tuple(sched_n), tuple(sched_e), s_e, s_n)
    if key not in _CACHE:
        _CACHE[key] = _build_nc(sched_n, sched_e, n_slabs, s_e, s_n)
    nc = _CACHE[key]

    if sim:
        from concourse.bass_interp import MultiCoreSim
        msim = MultiCoreSim(nc, num_cores=N_CORES)
        for c in range(N_CORES):
            cs = msim.cores[c]
            for kk, vv in in_maps[c].items():
                cs.tensor(kk)[:] = vv
        msim.simulate(check_with_hw=False)
        outT = np.array(msim.cores[0].tensor("out"))
        res = None
    else:
        res = run_bass_kernel_spmd(nc, in_maps, core_ids=list(range(N_CORES)),
                                   trace=trace)
        outT = res.results[0]["out"]  # [32, 1024] in permuted graph order

    outP = np.asarray(outT).T.astype(np.float32)   # [1024(perm), 32]
    outF = np.empty_like(outP)
    outF[p_global] = outP
    return np.ascontiguousarray(outF), res


def kernel(**inputs) -> np.ndarray:
    out, _ = run(inputs, trace=False)
    return out
